# revision 2
# baseline (speedup 1.0000x reference)
"""EntNetHead Trainium2 kernel.

Data-parallel over batch B=64 across 8 NeuronCores (8 batch rows per core);
T=256 recurrent steps run on-chip per core.

Per-core layout:
  - packed elementwise layout [128, 384]: partitions 0..39 hold rows
    (k*8+b) for hidden dims 0:384, partitions 64..103 the same rows for
    hidden dims 384:768.  Stationary operands are zero-padded to 64
    columns so every matmul writes partitions 0:64 / 64:128 and no engine
    ever reads uninitialized PSUM.  The two halves of h@U run as
    concurrent col-tiled matmuls (array cols 0 and 64).
  - hT [128, 6*64] fp16: transposed h per H-chunk -- the PE stationary.
  - fp16 data, fp32 PSUM accumulation / reductions.

Per step:
  PE:   z' = h@U + keys@V (bcast) + s_t@W (bcast)   (PSUM [128,384])
        gate Gram s_t.h (+ s_t.keys via sel matmul); score Gram q_{t-1}.h_t
        6 transposes h_new -> hT
  ACT:  cand = g * prelu(z') (Prelu, per-partition scale=g), sigmoid,
        Square+accum (norm^2)
  DVE:  gate mask-reduce, h_upd = h + cand, Newton rsqrt, h_new scale,
        score mask-reduce, hT PSUM->SBUF copy
"""

import sys

sys.path.insert(0, "/opt/trn_rl_repo")

from contextlib import ExitStack

import numpy as np

import concourse.bacc as bacc
import concourse.bass as bass
import concourse.tile as tile
from concourse import mybir
from concourse.bass_utils import run_bass_kernel_spmd

F32 = mybir.dt.float32
F16 = mybir.dt.float16
I32 = mybir.dt.int32
ALU = mybir.AluOpType
ACTF = mybir.ActivationFunctionType

T, B, H, K, L = 256, 64, 768, 5, 3
NC = 8
BL = B // NC          # 8 batch rows per core
R = K * BL            # 40 (k,b) rows
RP = 64               # padded stationary width
HC = H // 128         # 6 contraction chunks
HH = H // 2           # 384


def _host_consts():
    selK = np.zeros((K, RP), np.float16)
    for k in range(K):
        selK[k, k * BL:(k + 1) * BL] = 1.0
    selB = np.zeros((128, 16 * RP), np.float16)
    for m in range(16):
        for b in range(BL):
            for k in range(K):
                selB[m * BL + b, m * RP + k * BL + b] = 1.0
    maskG = np.zeros((128, BL), np.float16)
    for p in range(128):
        maskG[p, p % BL] = 1.0
    I64 = np.zeros((128, RP), np.float16)
    for j in range(RP):
        I64[j, j] = 1.0
        I64[64 + j, j] = 1.0
    I128 = np.eye(128, dtype=np.float32)
    selK32 = selK.astype(np.float32)
    mask24 = np.zeros((R, BL * L), np.float32)
    for k in range(K):
        for b in range(BL):
            mask24[k * BL + b, b * L:(b + 1) * L] = 1.0
    ones1x128 = np.ones((1, 128), np.float32)
    P64 = np.zeros((128, 128), np.float32)
    for i in range(128):
        P64[i, i % 64] = 1.0
        P64[i, i % 64 + 64] = 1.0
    return {
        "c_selK": selK, "c_selB32": selB, "c_maskG": maskG,
        "c_I64": I64, "c_I128": I128, "c_selK32": selK32,
        "c_mask24": mask24, "c_ones": ones1x128, "c_P64": P64,
        "c_I128h": np.eye(128, dtype=np.float16),
    }


class _StopBuild(Exception):
    pass


def _build(nsteps, use_prelu=True, debug=False, stop_after=None, loop_parts=7):
    nc = bacc.Bacc("TRN2", target_bir_lowering=False, debug=False)
    NR = nsteps * BL      # feature rows per core
    ntt = (NR + 127) // 128

    d_fs = nc.dram_tensor("features_sentence", [nsteps, BL, H], F32, kind="ExternalInput")
    d_fe = nc.dram_tensor("features_entity", [nsteps, BL, H], F32, kind="ExternalInput")
    d_keys = nc.dram_tensor("keys", [K, H], F32, kind="ExternalInput")
    d_U = nc.dram_tensor("U", [H, H], F32, kind="ExternalInput")
    d_V = nc.dram_tensor("V", [H, H], F32, kind="ExternalInput")
    d_W = nc.dram_tensor("W", [H, H], F32, kind="ExternalInput")
    d_alpha = nc.dram_tensor("alpha", [1], F32, kind="ExternalInput")
    d_Wout = nc.dram_tensor("W_out", [K, L], F32, kind="ExternalInput")
    d_bout = nc.dram_tensor("b_out", [L], F32, kind="ExternalInput")
    d_selK = nc.dram_tensor("c_selK", [K, RP], F16, kind="ExternalInput")
    d_selB32 = nc.dram_tensor("c_selB32", [128, 16 * RP], F16, kind="ExternalInput")
    d_maskG = nc.dram_tensor("c_maskG", [128, BL], F16, kind="ExternalInput")
    d_I64 = nc.dram_tensor("c_I64", [128, RP], F16, kind="ExternalInput")
    d_I128 = nc.dram_tensor("c_I128", [128, 128], F32, kind="ExternalInput")
    d_selK32 = nc.dram_tensor("c_selK32", [K, RP], F32, kind="ExternalInput")
    d_mask24 = nc.dram_tensor("c_mask24", [R, BL * L], F32, kind="ExternalInput")
    d_ones = nc.dram_tensor("c_ones", [1, 128], F32, kind="ExternalInput")
    d_P64 = nc.dram_tensor("c_P64", [128, 128], F32, kind="ExternalInput")
    d_I128h = nc.dram_tensor("c_I128h", [128, 128], F16, kind="ExternalInput")
    d_out = nc.dram_tensor("preds", [NR, L], F32, kind="ExternalOutput")
    if debug:
        d_dh = nc.dram_tensor("dbg_h", [nsteps, 128, HH], F16, kind="ExternalOutput")
        d_dg = nc.dram_tensor("dbg_g", [nsteps, 128], F32, kind="ExternalOutput")
        d_dss = nc.dram_tensor("dbg_ss", [nsteps, 128], F32, kind="ExternalOutput")
        d_dz = nc.dram_tensor("dbg_z", [nsteps, 128, HH], F32, kind="ExternalOutput")
        d_dhT = nc.dram_tensor("dbg_hT", [nsteps, 128, HC * RP], F16, kind="ExternalOutput")
        d_dhu = nc.dram_tensor("dbg_hu", [nsteps, 128, HH], F16, kind="ExternalOutput")
        d_drn = nc.dram_tensor("dbg_rn", [nsteps, 128], F32, kind="ExternalOutput")

    with tile.TileContext(nc) as tc, ExitStack() as ctx:
      try:
        ep = ctx.enter_context

        p_sT = ep(tc.tile_pool(name="sT", bufs=1))
        p_qT = ep(tc.tile_pool(name="qT", bufs=1))
        p_sW = ep(tc.tile_pool(name="sW", bufs=1))
        p_prm = ep(tc.tile_pool(name="prm", bufs=1))
        p_h = ep(tc.tile_pool(name="h", bufs=2))
        p_hT = ep(tc.tile_pool(name="hT", bufs=2))
        p_e16 = ep(tc.tile_pool(name="e16", bufs=2))
        p_sml = ep(tc.tile_pool(name="sml", bufs=3))

        sT = p_sT.tile([128, HC * NR], F16)   # [h%128, c*NR + t*8+b]
        qT = p_qT.tile([128, HC * NR], F16)
        sW = p_sW.tile([128, ntt * H], F16)   # [row%128, tile*768+h]
        U16 = p_prm.tile([128, HC * H], F16, tag="U16")
        keyV = p_prm.tile([128, H], F16, tag="keyV")
        keys16 = p_prm.tile([128, H], F16, tag="keys16")
        keysT = p_prm.tile([128, HC * 8], F16, tag="keysT")
        sK = p_prm.tile([128, NR], F16, tag="sK")
        selK16 = p_prm.tile([128, RP], F16, tag="selK16")
        selB32 = p_prm.tile([128, 16 * RP], F16, tag="selB32")
        maskG = p_prm.tile([128, BL], F16, tag="maskG")
        I64 = p_prm.tile([128, RP], F16, tag="I64")
        I128 = p_prm.tile([128, 128], F32, tag="I128")
        selK32 = p_prm.tile([128, RP], F32, tag="selK32")
        mask24 = p_prm.tile([128, BL * L], F32, tag="mask24")
        ones_r = p_prm.tile([128, 128], F32, tag="ones_r")
        P64 = p_prm.tile([128, 128], F32, tag="P64")
        I128h = p_prm.tile([128, 128], F16, tag="I128h")
        alphav = p_prm.tile([128, 1], F32, tag="alphav")
        av_n = p_prm.tile([128, 1], F32, tag="av_n")
        av_p = p_prm.tile([128, 1], F32, tag="av_p")
        Wsel = p_prm.tile([128, BL * L], F32, tag="Wsel")
        bvec = p_prm.tile([128, 1], F32, tag="bvec")
        scores = p_prm.tile([128, nsteps], F32, tag="scores")

        dma = nc.sync.dma_start

        dma(selK16[0:K, :], d_selK.ap())
        dma(selB32[:, :], d_selB32.ap())
        dma(maskG[:, :], d_maskG.ap())
        dma(I64[:, :], d_I64.ap())
        dma(I128[:, :], d_I128.ap())
        dma(selK32[0:K, :], d_selK32.ap())
        dma(mask24[0:R, :], d_mask24.ap())
        dma(ones_r[0:1, :], d_ones.ap())
        dma(P64[:, :], d_P64.ap())
        dma(I128h[:, :], d_I128h.ap())
        for b in range(BL):
            dma(bvec[b * L:(b + 1) * L, 0:1], bass.AP(d_bout, 0, [[1, L], [1, 1]]))

        def hts(hT_tile, c):
            if c < 3:
                return hT_tile[:, c * 128:c * 128 + RP]
            return hT_tile[:, (c - 3) * 128 + RP:(c - 2) * 128]

        def emit_transposes(h_tile, out_psum):
            for cc in range(3):
                nc.tensor.transpose(out_psum[:, cc * 128:(cc + 1) * 128],
                                    h_tile[:, cc * 128:(cc + 1) * 128],
                                    I128h[:, :])

        with tc.tile_pool(name="pr32", bufs=2) as p32, \
             tc.tile_pool(name="prps", bufs=4, space="PSUM") as pps:

            def ptile(shape, dt):
                return pps.tile(shape, dt, tag="ps", name="ps")

            # keys
            kn = p32.tile([128, H], F32, tag="kn")
            dma(kn[0:K, :], d_keys.ap())
            nc.vector.tensor_copy(keys16[0:K, :], kn[0:K, :])
            # U
            un = p32.tile([128, HC * H], F32, tag="big")
            for c in range(HC):
                dma(un[:, c * H:(c + 1) * H], d_U.ap()[c * 128:(c + 1) * 128, :])
            nc.vector.tensor_copy(U16[:, :], un[:, :])
            # keysT via PE transpose
            tps = ptile([128, HC * 8], F16)
            for c in range(HC):
                nc.tensor.transpose(tps[:, c * 8:c * 8 + K],
                                    keys16[0:K, c * 128:(c + 1) * 128],
                                    I64[0:K, 0:K])
            for c in range(HC):
                nc.vector.tensor_copy(keysT[:, c * 8:c * 8 + K],
                                      tps[:, c * 8:c * 8 + K])
            # V -> keyV = keys @ V
            vn = p32.tile([128, HC * H], F32, tag="big")
            for c in range(HC):
                dma(vn[:, c * H:(c + 1) * H], d_V.ap()[c * 128:(c + 1) * 128, :])
            v16 = p32.tile([128, HC * H], F16, tag="big16")
            nc.vector.tensor_copy(v16[:, :], vn[:, :])
            for i in range(2):
                kvp = ptile([128, HH], F32)
                for c in range(HC):
                    nc.tensor.matmul(kvp[0:K, :],
                                     keysT[:, c * 8:c * 8 + K],
                                     v16[:, c * H + i * HH: c * H + (i + 1) * HH],
                                     start=(c == 0), stop=(c == HC - 1))
                nc.vector.tensor_copy(keyV[0:K, i * HH:(i + 1) * HH], kvp[0:K, :])
            # W (fp16) for sW matmuls
            wn = p32.tile([128, HC * H], F32, tag="big")
            for c in range(HC):
                dma(wn[:, c * H:(c + 1) * H], d_W.ap()[c * 128:(c + 1) * 128, :])
            w16 = p32.tile([128, HC * H], F16, tag="big16w")
            nc.vector.tensor_copy(w16[:, :], wn[:, :])

            # alpha -> all partitions
            asb = p32.tile([128, 1], F32, tag="asb")
            dma(asb[0:1, 0:1], bass.AP(d_alpha, 0, [[1, 1], [1, 1]]))
            alp = ptile([128, 1], F32)
            nc.tensor.matmul(alp[:, :], ones_r[0:1, :], asb[0:1, 0:1],
                             start=True, stop=True)
            nc.vector.tensor_copy(alphav[:, :], alp[:, :])
            nc.vector.tensor_scalar(av_n[:, :], alphav[:, :], -0.5, 0.5,
                                    ALU.mult, ALU.add)
            nc.vector.tensor_scalar(av_p[:, :], alphav[:, :], 0.5, 0.5,
                                    ALU.mult, ALU.add)

            # W_out -> Wsel
            wo = p32.tile([128, L], F32, tag="wo")
            dma(wo[0:K, :], d_Wout.ap())
            wrp = ptile([128, L], F32)
            nc.tensor.matmul(wrp[0:R, :], selK32[0:K, 0:R], wo[0:K, :],
                             start=True, stop=True)
            wrs = p32.tile([128, L], F32, tag="wrs")
            nc.vector.tensor_copy(wrs[0:R, :], wrp[0:R, :])
            for b in range(BL):
                nc.vector.tensor_mul(Wsel[0:R, b * L:(b + 1) * L],
                                     wrs[0:R, :], mask24[0:R, b * L:(b + 1) * L])

            # features: DMA, transpose (and sW for the sentence stream)
            def do_feat(dram, dstT, with_sw):
                for tb in range(ntt):
                    fn = p32.tile([128, H], F32, tag="fnat")
                    nrow = min(128, NR - tb * 128)
                    src = bass.AP(dram, tb * 128 * H, [[H, nrow], [1, H]])
                    dma(fn[0:nrow, :], src)
                    for grp in range(2):
                        tp = ptile([128, 3 * 128], F32)
                        for j in range(3):
                            c = grp * 3 + j
                            nc.tensor.transpose(
                                tp[:, j * nrow:(j + 1) * nrow],
                                fn[0:nrow, c * 128:(c + 1) * 128],
                                I128[0:nrow, 0:nrow])
                        dst = bass.AP(
                            dstT.tensor,
                            dstT.offset + (grp * 3) * NR + tb * 128,
                            [[HC * NR, 128], [NR, 3], [1, nrow]])
                        nc.vector.tensor_copy(
                            dst,
                            tp[:, 0:3 * nrow].rearrange("p (a b) -> p a b", a=3))
                    if with_sw:
                        for i in range(2):
                            swp = ptile([128, HH], F32)
                            for c in range(HC):
                                lhs = dstT[:, c * NR + tb * 128:
                                           c * NR + tb * 128 + nrow]
                                nc.tensor.matmul(
                                    swp[0:nrow, :], lhs,
                                    w16[:, c * H + i * HH:c * H + (i + 1) * HH],
                                    start=(c == 0), stop=(c == HC - 1))
                            nc.vector.tensor_copy(
                                sW[0:nrow, tb * H + i * HH:tb * H + (i + 1) * HH],
                                swp[0:nrow, :])

            if NR < 128:
                nc.vector.memset(sW[:, :], 0.0)
            if stop_after is None or stop_after >= 2:
                do_feat(d_fs, sT, True)
                do_feat(d_fe, qT, False)

            # sK = keys @ s^T  [5, NR]
            for q in range((NR + 511) // 512) if (stop_after is None or (stop_after >= 3 and stop_after != 35)) else []:
                ncol = min(512, NR - q * 512)
                skp = ptile([128, 512], F32)
                for c in range(HC):
                    nc.tensor.matmul(
                        skp[0:K, 0:ncol], keysT[:, c * 8:c * 8 + K],
                        sT[:, c * NR + q * 512:c * NR + q * 512 + ncol],
                        start=(c == 0), stop=(c == HC - 1))
                nc.vector.tensor_copy(sK[0:K, q * 512:q * 512 + ncol],
                                      skp[0:K, 0:ncol])

            if stop_after is not None and stop_after < 3:
                raise _StopBuild()
            # h0 = keys broadcast to (k,b) rows (zero into pad rows)
            h0p = ptile([128, HH], F32)
            nc.tensor.matmul(h0p[0:RP, :], selK16[0:K, :], keys16[0:K, 0:HH],
                             start=True, stop=True, skip_group_check=True)
            if stop_after == 31:
                raise _StopBuild()
            nc.tensor.matmul(h0p[64:128, :], selK16[0:K, :], keys16[0:K, HH:H],
                             start=True, stop=True, skip_group_check=True)
            h_cur = p_h.tile([128, HH], F16, tag="h")
            nc.vector.tensor_copy(h_cur[:, :], h0p[:, :])
            if stop_after == 32:
                raise _StopBuild()

            tp0 = ptile([128, 3 * 128], F16)
            emit_transposes(h_cur, tp0)
            hT_cur = p_hT.tile([128, 3 * 128], F16, tag="hT")
            if stop_after in (33, 35):
                raise _StopBuild()
            nc.vector.tensor_copy(hT_cur[:, :], tp0[:, :])

        # ---- main loop ----
        p_zps = ep(tc.tile_pool(name="zps", bufs=2, space="PSUM"))
        p_gps = ep(tc.tile_pool(name="gps", bufs=2, space="PSUM"))
        p_qps = ep(tc.tile_pool(name="qps", bufs=2, space="PSUM"))
        p_tps = ep(tc.tile_pool(name="tps", bufs=2, space="PSUM"))

        def qgram(hT_tile, tq, qps):
            for c in range(HC):
                nc.tensor.matmul(
                    qps[0:RP, 0:BL], hts(hT_tile, c),
                    qT[:, c * NR + tq * BL:c * NR + (tq + 1) * BL],
                    start=(c == 0), stop=(c == HC - 1))

        def qscore(qps, tq):
            qsc = p_sml.tile([128, BL], F32, tag="qsc")
            nc.vector.tensor_mul(qsc[0:RP, :], qps[0:RP, 0:BL], maskG[0:RP, :])
            nc.vector.tensor_reduce(scores[0:RP, tq:tq + 1], qsc[0:RP, :],
                                    mybir.AxisListType.X, ALU.add)

        if stop_after is not None and stop_after < 4:
            raise _StopBuild()
        for t in range(nsteps):
            zP = p_zps.tile([128, HH], F32, tag="z")
            gP = p_gps.tile([128, BL], F32, tag="g")

            tb, m = (t * BL) // 128, t % 16
            nc.tensor.matmul(zP[0:RP, :], selK16[0:K, :], keyV[0:K, 0:HH],
                             start=True, stop=False, skip_group_check=True)
            nc.tensor.matmul(zP[64:128, :], selK16[0:K, :], keyV[0:K, HH:H],
                             start=True, stop=False, skip_group_check=True)
            lsel = selB32[:, m * RP:(m + 1) * RP]
            nc.tensor.matmul(zP[0:RP, :], lsel,
                             sW[:, tb * H:tb * H + HH],
                             start=False, stop=False, skip_group_check=True)
            nc.tensor.matmul(zP[64:128, :], lsel,
                             sW[:, tb * H + HH:(tb + 1) * H],
                             start=False, stop=False, skip_group_check=True)
            nc.tensor.matmul(gP[0:RP, 0:BL], selK16[0:K, :],
                             sK[0:K, t * BL:(t + 1) * BL], start=True, stop=False, skip_group_check=True)
            nc.tensor.matmul(gP[64:128, 0:BL], selK16[0:K, :],
                             sK[0:K, t * BL:(t + 1) * BL], start=True, stop=False, skip_group_check=True)
            if t > 0:
                qP = p_qps.tile([128, BL], F32, tag="q")
                qgram(hT_cur, t - 1, qP)
            for c in range(HC):
                lhs = hts(hT_cur, c)
                st_ = sT[:, c * NR + t * BL:c * NR + (t + 1) * BL]
                nc.tensor.matmul(zP[0:RP, :], lhs, U16[:, c * H:c * H + HH],
                                 start=False, stop=(c == HC - 1), skip_group_check=True)
                nc.tensor.matmul(zP[64:128, :], lhs,
                                 U16[:, c * H + HH:(c + 1) * H],
                                 start=False, stop=(c == HC - 1), skip_group_check=True)
                nc.tensor.matmul(gP[0:RP, 0:BL], lhs, st_,
                                 start=False, stop=(c == HC - 1), skip_group_check=True)
                nc.tensor.matmul(gP[64:128, 0:BL], lhs, st_,
                                 start=False, stop=(c == HC - 1), skip_group_check=True)
            if loop_parts < 2:
                continue
            # gate: masked free-axis reduce, then sigmoid
            gsc = p_sml.tile([128, BL], F32, tag="gsc")
            gpre = p_sml.tile([128, 1], F32, tag="gpre")
            nc.vector.tensor_mul(gsc[:, :], gP[:, 0:BL], maskG[:, :])
            nc.vector.tensor_reduce(gpre[:, :], gsc[:, :],
                                    mybir.AxisListType.X, ALU.add)
            gsig = p_sml.tile([128, 1], F32, tag="gsig")
            nc.scalar.activation(gsig[:, :], gpre[:, :], ACTF.Sigmoid)
            if loop_parts < 3:
                continue
            # h_upd = h + g * prelu(z')
            hu = p_e16.tile([128, HH], F16, tag="hu")
            if use_prelu:
                cand = p_e16.tile([128, HH], F16, tag="cand")
                nc.scalar.activation(cand[:, :], zP[:, :], ACTF.Prelu,
                                     scale=gsig[:, :], alpha=alphav[:, :])
                nc.vector.tensor_add(hu[:, :], h_cur[:, :], cand[:, :])
            else:
                # g*prelu(z) = g(1+a)/2 * z + g(1-a)/2 * |z|
                ca = p_sml.tile([128, 1], F32, tag="ca")
                cb = p_sml.tile([128, 1], F32, tag="cb")
                nc.vector.tensor_mul(ca[:, :], gsig[:, :], av_n[:, :])
                nc.vector.tensor_mul(cb[:, :], gsig[:, :], av_p[:, :])
                cand = p_e16.tile([128, HH], F16, tag="cand")
                nc.scalar.activation(cand[:, :], zP[:, :], ACTF.Abs,
                                     scale=ca[:, :])
                bv = p_e16.tile([128, HH], F16, tag="bv")
                nc.vector.tensor_scalar(bv[:, :], zP[:, :], cb[:, :], None,
                                        ALU.mult)
                nc.vector.tensor_add(hu[:, :], h_cur[:, :], cand[:, :])
                nc.vector.tensor_add(hu[:, :], hu[:, :], bv[:, :])
            squ = p_e16.tile([128, HH], F16, tag="squ")
            ss = p_sml.tile([128, 1], F32, tag="ss")
            nc.scalar.activation(squ[:, :], hu[:, :], ACTF.Square,
                                 accum_out=ss[:, :])
            if loop_parts < 4:
                continue
            # ss2[p] = ss[p] + ss[p^64]  (norm spans both packed halves)
            ssp = p_tps.tile([128, 1], F32, tag="t", name="ssp")
            nc.tensor.matmul(ssp[:, :], P64[:, :], ss[:, :], start=True, stop=True)
            if loop_parts < 5:
                continue
            # rn = rsqrt(ss2)
            sdi = p_sml.tile([128, 1], I32, tag="sdi")
            nc.vector.tensor_scalar(sdi[:, :], ssp.bitcast(I32)[:, :], 1, None,
                                    ALU.logical_shift_right)
            nc.vector.tensor_scalar(sdi[:, :], sdi[:, :], -1, 0x5F3759DF,
                                    ALU.mult, ALU.add)
            rn = sdi.bitcast(F32)
            ra = p_sml.tile([128, 1], F32, tag="ra")
            for _ in range(2):
                nc.vector.tensor_mul(ra[:, :], rn[:, :], ssp[:, :])
                nc.vector.tensor_mul(ra[:, :], ra[:, :], rn[:, :])
                nc.vector.tensor_scalar(ra[:, :], ra[:, :], -0.5, 1.5,
                                        ALU.mult, ALU.add)
                nc.vector.tensor_mul(rn[:, :], rn[:, :], ra[:, :])
            h_new = p_h.tile([128, HH], F16, tag="h")
            nc.vector.tensor_scalar(h_new[:, :], hu[:, :], rn[:, :], None,
                                    ALU.mult)
            if loop_parts < 6:
                continue
            if t > 0:
                qscore(qP, t - 1)
            if loop_parts < 7:
                continue
            tP = p_tps.tile([128, 3 * 128], F16, tag="t")
            emit_transposes(h_new, tP)
            hT_new = p_hT.tile([128, 3 * 128], F16, tag="hT")
            nc.vector.tensor_copy(hT_new[:, :], tP[:, :])
            if debug:
                zs = p_e16.tile([128, HH], F32, tag="zs", name="zs")
                nc.scalar.copy(zs[:, :], zP[:, :])
                dma(bass.AP(d_dz, t * 128 * HH, [[HH, 128], [1, HH]]), zs[:, :])
                dma(bass.AP(d_dh, t * 128 * HH, [[HH, 128], [1, HH]]), h_new[:, :])
                dma(bass.AP(d_dg, t * 128, [[1, 128], [1, 1]]), gsig[:, :])
                dma(bass.AP(d_dss, t * 128, [[1, 128], [1, 1]]), ss[:, :])
                dma(bass.AP(d_dhT, t * 128 * HC * RP, [[HC * RP, 128], [1, HC * RP]]), hT_new[:, :])
                dma(bass.AP(d_dhu, t * 128 * HH, [[HH, 128], [1, HH]]), hu[:, :])
                dma(bass.AP(d_drn, t * 128, [[1, 128], [1, 1]]), rn[:, :])
            h_cur, hT_cur = h_new, hT_new

        if (stop_after is not None and stop_after < 5) or loop_parts < 7:
            raise _StopBuild()
        # epilogue: last score + output head
        qPf = p_qps.tile([128, BL], F32, tag="q")
        qgram(hT_cur, nsteps - 1, qPf)
        qscore(qPf, nsteps - 1)

        pP = p_qps.tile([128, nsteps], F32, tag="q")
        nc.tensor.matmul(pP[0:BL * L, :], Wsel[0:R, 0:BL * L],
                         scores[0:R, 0:nsteps], start=True, stop=True)
        osb = p_prm.tile([128, nsteps], F32, tag="osb")
        nc.vector.tensor_scalar(osb[0:BL * L, :], pP[0:BL * L, :],
                                bvec[0:BL * L, :], None, ALU.add)
        nc.sync.dma_start(bass.AP(d_out, 0, [[1, BL * L], [BL * L, nsteps]]),
                          osb[0:BL * L, :])
      except _StopBuild:
        with tc.tile_pool(name="dummy", bufs=1) as dp:
            dt_ = dp.tile([128, L], F32)
            nc.vector.memset(dt_[:, :], 0.0)
            for i in range(ntt):
                nr = min(128, NR - i * 128)
                nc.sync.dma_start(
                    bass.AP(d_out, i * 128 * L, [[L, nr], [1, L]]), dt_[0:nr, :])

    nc.compile()
    return nc


_CACHE = {}


def _get(nsteps):
    if nsteps not in _CACHE:
        _CACHE[nsteps] = _build(nsteps)
    return _CACHE[nsteps]


def run(inputs, **spmd_kwargs):
    nsteps = inputs["features_sentence"].shape[0]
    nc = _get(nsteps)
    consts = _host_consts()
    fs = np.ascontiguousarray(np.asarray(inputs["features_sentence"], dtype=np.float32))
    fe = np.ascontiguousarray(np.asarray(inputs["features_entity"], dtype=np.float32))
    shared = {k: np.ascontiguousarray(np.asarray(inputs[k], dtype=np.float32))
              for k in ("keys", "U", "V", "W", "alpha", "W_out", "b_out")}
    shared.update(consts)
    in_maps = []
    for c in range(NC):
        m = dict(shared)
        m["features_sentence"] = np.ascontiguousarray(fs[:, c * BL:(c + 1) * BL, :])
        m["features_entity"] = np.ascontiguousarray(fe[:, c * BL:(c + 1) * BL, :])
        in_maps.append(m)
    res = run_bass_kernel_spmd(nc, in_maps, core_ids=list(range(NC)), **spmd_kwargs)
    outs = [r["preds"].reshape(nsteps, BL, L) for r in res.results]
    return np.concatenate(outs, axis=1).reshape(nsteps * B, L), res


def kernel(**inputs):
    out, _ = run(inputs)
    return out



# revision 11
# speedup vs baseline: 1.3643x; 1.3643x over previous
"""EntNetHead Trainium2 kernel (v2).

Data-parallel over batch B=64 across 8 NeuronCores (8 batch rows per core);
T=256 recurrent steps run on-chip per core.

Per-core layout:
  - packed elementwise layout [128, 384]: partitions 0..39 hold rows
    (k*8+b) for hidden dims 0:384, partitions 64..103 the same rows for
    hidden dims 384:768.
  - hT [128, 3*128] fp16: transposed h per 128-chunk (PE stationary).
  - sqT [128, 6*(2*NR+16)] fp16: per chunk c, step slot t holds 16 cols
    [q_{t-1} (8) | s_t (8)] so gate+score grams share one matmul.
  - fp16 data, fp32 PSUM accumulation / reductions.

Per step:
  PE:  grams (3 MM N=16 per half, chunk stationaries), sK inject,
       z' = h@U + keys@V + s@W (6+2 MM N=384 per half),
       P64 cross-half sums, 3 transposes, filler MMs to pin HAM warm.
  ACT: sigmoid(gate), cand = prelu(z')*g, sqrt(ss)
  DVE: mask-reduce, hu = h + cand, square+reduce, reciprocal, h scale,
       hT copy, scores copy
"""

import sys

sys.path.insert(0, "/opt/trn_rl_repo")

from contextlib import ExitStack

import numpy as np

import concourse.bacc as bacc
import concourse.bass as bass
import concourse.tile as tile
from concourse import mybir
from concourse.bass_utils import run_bass_kernel_spmd

F32 = mybir.dt.float32
F16 = mybir.dt.float16
ALU = mybir.AluOpType
ACTF = mybir.ActivationFunctionType
AXX = mybir.AxisListType.X

T, B, H, K, L = 256, 64, 768, 5, 3
NC = 8
BL = B // NC          # 8 batch rows per core
R = K * BL            # 40 (k,b) rows
RP = 64               # padded stationary width
HC = H // 128         # 6 contraction chunks
HH = H // 2           # 384


def _host_consts():
    selK = np.zeros((K, RP), np.float16)
    for k in range(K):
        selK[k, k * BL:(k + 1) * BL] = 1.0
    selB = np.zeros((128, 16 * RP), np.float16)
    for m in range(16):
        for b in range(BL):
            for k in range(K):
                selB[m * BL + b, m * RP + k * BL + b] = 1.0
    maskQG = np.zeros((128, 16), np.float16)
    for p in range(128):
        maskQG[p, p % BL] = 1.0
        maskQG[p, 8 + p % BL] = 1.0
    I64 = np.zeros((128, RP), np.float16)
    for j in range(RP):
        I64[j, j] = 1.0
        I64[64 + j, j] = 1.0
    I128 = np.eye(128, dtype=np.float32)
    selK32 = selK.astype(np.float32)
    mask24 = np.zeros((R, BL * L), np.float32)
    for k in range(K):
        for b in range(BL):
            mask24[k * BL + b, b * L:(b + 1) * L] = 1.0
    ones1x128 = np.ones((1, 128), np.float32)
    P64 = np.zeros((128, 128), np.float32)
    for i in range(128):
        P64[i, i % 64] = 1.0
        P64[i, i % 64 + 64] = 1.0
    return {
        "c_selK": selK, "c_selB32": selB, "c_maskQG": maskQG,
        "c_I64": I64, "c_I128": I128, "c_selK32": selK32,
        "c_mask24": mask24, "c_ones": ones1x128, "c_P64": P64,
        "c_I128h": np.eye(128, dtype=np.float16),
    }


def _build(nsteps, n_fill=3):
    nc = bacc.Bacc("TRN2", target_bir_lowering=False, debug=False)
    NR = nsteps * BL      # feature rows per core
    ntt = (NR + 127) // 128
    BLK = 2 * NR + 16     # sqT cols per chunk: [q_{t-1}|s_t] slots + final q

    d_fs = nc.dram_tensor("features_sentence", [nsteps, BL, H], F32, kind="ExternalInput")
    d_fe = nc.dram_tensor("features_entity", [nsteps, BL, H], F32, kind="ExternalInput")
    d_keys = nc.dram_tensor("keys", [K, H], F32, kind="ExternalInput")
    d_U = nc.dram_tensor("U", [H, H], F32, kind="ExternalInput")
    d_V = nc.dram_tensor("V", [H, H], F32, kind="ExternalInput")
    d_W = nc.dram_tensor("W", [H, H], F32, kind="ExternalInput")
    d_alpha = nc.dram_tensor("alpha", [1], F32, kind="ExternalInput")
    d_Wout = nc.dram_tensor("W_out", [K, L], F32, kind="ExternalInput")
    d_bout = nc.dram_tensor("b_out", [L], F32, kind="ExternalInput")
    d_selK = nc.dram_tensor("c_selK", [K, RP], F16, kind="ExternalInput")
    d_selB32 = nc.dram_tensor("c_selB32", [128, 16 * RP], F16, kind="ExternalInput")
    d_maskQG = nc.dram_tensor("c_maskQG", [128, 16], F16, kind="ExternalInput")
    d_I64 = nc.dram_tensor("c_I64", [128, RP], F16, kind="ExternalInput")
    d_I128 = nc.dram_tensor("c_I128", [128, 128], F32, kind="ExternalInput")
    d_selK32 = nc.dram_tensor("c_selK32", [K, RP], F32, kind="ExternalInput")
    d_mask24 = nc.dram_tensor("c_mask24", [R, BL * L], F32, kind="ExternalInput")
    d_ones = nc.dram_tensor("c_ones", [1, 128], F32, kind="ExternalInput")
    d_P64 = nc.dram_tensor("c_P64", [128, 128], F32, kind="ExternalInput")
    d_I128h = nc.dram_tensor("c_I128h", [128, 128], F16, kind="ExternalInput")
    d_out = nc.dram_tensor("preds", [NR, L], F32, kind="ExternalOutput")

    with tile.TileContext(nc) as tc, ExitStack() as ctx:
        ep = ctx.enter_context

        p_sqT = ep(tc.tile_pool(name="sqT", bufs=1))
        p_sW = ep(tc.tile_pool(name="sW", bufs=1))
        p_prm = ep(tc.tile_pool(name="prm", bufs=1))
        p_h = ep(tc.tile_pool(name="h", bufs=2))
        p_hT = ep(tc.tile_pool(name="hT", bufs=2))
        p_e16 = ep(tc.tile_pool(name="e16", bufs=2))
        p_sml = ep(tc.tile_pool(name="sml", bufs=3))

        sqT = p_sqT.tile([128, HC * BLK], F16)
        sW = p_sW.tile([128, ntt * H], F16)   # [row%128, tile*768+h]
        U16 = p_prm.tile([128, HC * H], F16, tag="U16")
        keyV = p_prm.tile([128, H], F16, tag="keyV")
        keys16 = p_prm.tile([128, H], F16, tag="keys16")
        keysT = p_prm.tile([128, HC * 8], F16, tag="keysT")
        sK = p_prm.tile([128, NR], F16, tag="sK")
        selK16 = p_prm.tile([128, RP], F16, tag="selK16")
        selB32 = p_prm.tile([128, 16 * RP], F16, tag="selB32")
        maskQG = p_prm.tile([128, 16], F16, tag="maskQG")
        I64 = p_prm.tile([128, RP], F16, tag="I64")
        I128 = p_prm.tile([128, 128], F32, tag="I128")
        selK32 = p_prm.tile([128, RP], F32, tag="selK32")
        mask24 = p_prm.tile([128, BL * L], F32, tag="mask24")
        ones_r = p_prm.tile([128, 128], F32, tag="ones_r")
        P64 = p_prm.tile([128, 128], F32, tag="P64")
        I128h = p_prm.tile([128, 128], F16, tag="I128h")
        alphav = p_prm.tile([128, 1], F32, tag="alphav")
        epsv = p_prm.tile([128, 1], F32, tag="epsv")
        Wsel = p_prm.tile([128, BL * L], F32, tag="Wsel")
        bvec = p_prm.tile([128, 1], F32, tag="bvec")
        scores = p_prm.tile([128, nsteps], F32, tag="scores")

        dma = nc.sync.dma_start

        dma(selK16[0:K, :], d_selK.ap())
        dma(selB32[:, :], d_selB32.ap())
        dma(maskQG[:, :], d_maskQG.ap())
        dma(I64[:, :], d_I64.ap())
        dma(I128[:, :], d_I128.ap())
        dma(selK32[0:K, :], d_selK32.ap())
        dma(mask24[0:R, :], d_mask24.ap())
        dma(ones_r[0:1, :], d_ones.ap())
        dma(P64[:, :], d_P64.ap())
        dma(I128h[:, :], d_I128h.ap())
        for b in range(BL):
            dma(bvec[b * L:(b + 1) * L, 0:1], bass.AP(d_bout, 0, [[1, L], [1, 1]]))
        nc.vector.memset(epsv[:, :], 1e-12)

        def hts(hT_tile, c):
            if c < 3:
                return hT_tile[:, c * 128:c * 128 + RP]
            return hT_tile[:, (c - 3) * 128 + RP:(c - 2) * 128]

        def emit_transposes(h_tile, out_psum):
            for cc in range(3):
                nc.tensor.transpose(out_psum[:, cc * 128:(cc + 1) * 128],
                                    h_tile[:, cc * 128:(cc + 1) * 128],
                                    I128h[:, :])

        with tc.tile_pool(name="pr32", bufs=1) as p32, \
             tc.tile_pool(name="prps", bufs=4, space="PSUM") as pps:

            def ptile(shape, dt):
                return pps.tile(shape, dt, tag="ps", name="ps")

            sT = p32.tile([128, HC * NR], F16, tag="sT", name="sT", bufs=1)

            # keys
            kn = p32.tile([128, H], F32, tag="kn")
            dma(kn[0:K, :], d_keys.ap())
            nc.vector.tensor_copy(keys16[0:K, :], kn[0:K, :])
            # U
            un = p32.tile([128, HC * H], F32, tag="big")
            for c in range(HC):
                dma(un[:, c * H:(c + 1) * H], d_U.ap()[c * 128:(c + 1) * 128, :])
            nc.vector.tensor_copy(U16[:, :], un[:, :])
            # keysT via PE transpose
            tps = ptile([128, HC * 8], F16)
            for c in range(HC):
                nc.tensor.transpose(tps[:, c * 8:c * 8 + K],
                                    keys16[0:K, c * 128:(c + 1) * 128],
                                    I64[0:K, 0:K])
            for c in range(HC):
                nc.vector.tensor_copy(keysT[:, c * 8:c * 8 + K],
                                      tps[:, c * 8:c * 8 + K])
            # V -> keyV = keys @ V
            vn = p32.tile([128, HC * H], F32, tag="big")
            for c in range(HC):
                dma(vn[:, c * H:(c + 1) * H], d_V.ap()[c * 128:(c + 1) * 128, :])
            v16 = p32.tile([128, HC * H], F16, tag="big16")
            nc.vector.tensor_copy(v16[:, :], vn[:, :])
            for i in range(2):
                kvp = ptile([128, HH], F32)
                for c in range(HC):
                    nc.tensor.matmul(kvp[0:K, :],
                                     keysT[:, c * 8:c * 8 + K],
                                     v16[:, c * H + i * HH: c * H + (i + 1) * HH],
                                     start=(c == 0), stop=(c == HC - 1))
                nc.vector.tensor_copy(keyV[0:K, i * HH:(i + 1) * HH], kvp[0:K, :])
            # W (fp16) for sW matmuls
            wn = p32.tile([128, HC * H], F32, tag="big")
            for c in range(HC):
                dma(wn[:, c * H:(c + 1) * H], d_W.ap()[c * 128:(c + 1) * 128, :])
            w16 = p32.tile([128, HC * H], F16, tag="big16w")
            nc.vector.tensor_copy(w16[:, :], wn[:, :])

            # alpha -> all partitions
            asb = p32.tile([128, 1], F32, tag="asb")
            dma(asb[0:1, 0:1], bass.AP(d_alpha, 0, [[1, 1], [1, 1]]))
            alp = ptile([128, 1], F32)
            nc.tensor.matmul(alp[:, :], ones_r[0:1, :], asb[0:1, 0:1],
                             start=True, stop=True)
            nc.vector.tensor_copy(alphav[:, :], alp[:, :])

            # W_out -> Wsel
            wo = p32.tile([128, L], F32, tag="wo")
            dma(wo[0:K, :], d_Wout.ap())
            wrp = ptile([128, L], F32)
            nc.tensor.matmul(wrp[0:R, :], selK32[0:K, 0:R], wo[0:K, :],
                             start=True, stop=True)
            wrs = p32.tile([128, L], F32, tag="wrs")
            nc.vector.tensor_copy(wrs[0:R, :], wrp[0:R, :])
            for b in range(BL):
                nc.vector.tensor_mul(Wsel[0:R, b * L:(b + 1) * L],
                                     wrs[0:R, :], mask24[0:R, b * L:(b + 1) * L])

            # zero q-slot 0 and the s-part of the final slot of sqT
            for c in range(HC):
                nc.vector.memset(sqT[:, c * BLK:c * BLK + 8], 0.0)
                nc.vector.memset(sqT[:, c * BLK + 2 * NR + 8:(c + 1) * BLK], 0.0)

            # features: DMA, transpose into sqT slots (and sW for sentence)
            def do_feat(dram, is_q):
                for tb in range(ntt):
                    fn = p32.tile([128, H], F32, tag="fnat")
                    nrow = min(128, NR - tb * 128)
                    src = bass.AP(dram, tb * 128 * H, [[H, nrow], [1, H]])
                    dma(fn[0:nrow, :], src)
                    for grp in range(2):
                        tp = ptile([128, 3 * 128], F32)
                        for j in range(3):
                            c = grp * 3 + j
                            nc.tensor.transpose(
                                tp[:, j * nrow:(j + 1) * nrow],
                                fn[0:nrow, c * 128:(c + 1) * 128],
                                I128[0:nrow, 0:nrow])
                        # slot col: q_t -> slot t+1 cols 0:8 ; s_t -> slot t cols 8:16
                        off = (grp * 3) * BLK + tb * 16 * 16 + (16 if is_q else 8)
                        dst = bass.AP(
                            sqT.tensor,
                            sqT.offset + off,
                            [[HC * BLK, 128], [BLK, 3], [16, 16], [1, BL]])
                        nc.vector.tensor_copy(
                            dst,
                            tp[:, 0:3 * nrow].rearrange(
                                "p (a b c) -> p a b c", a=3, b=16))
                        if not is_q:
                            dstc = bass.AP(
                                sT.tensor,
                                sT.offset + (grp * 3) * NR + tb * 128,
                                [[HC * NR, 128], [NR, 3], [1, nrow]])
                            nc.vector.tensor_copy(
                                dstc,
                                tp[:, 0:3 * nrow].rearrange(
                                    "p (a b) -> p a b", a=3))
                    if not is_q:
                        for i in range(2):
                            swp = ptile([128, HH], F32)
                            for c in range(HC):
                                lhs = sT[:, c * NR + tb * 128:
                                         c * NR + tb * 128 + nrow]
                                nc.tensor.matmul(
                                    swp[0:nrow, :], lhs,
                                    w16[:, c * H + i * HH:c * H + (i + 1) * HH],
                                    start=(c == 0), stop=(c == HC - 1))
                            nc.vector.tensor_copy(
                                sW[0:nrow, tb * H + i * HH:tb * H + (i + 1) * HH],
                                swp[0:nrow, :])

            do_feat(d_fs, False)
            do_feat(d_fe, True)

            # sK = keys @ s^T  [5, NR]
            for q in range((NR + 511) // 512):
                ncol = min(512, NR - q * 512)
                skp = ptile([128, 512], F32)
                for c in range(HC):
                    nc.tensor.matmul(
                        skp[0:K, 0:ncol], keysT[:, c * 8:c * 8 + K],
                        sT[:, c * NR + q * 512:c * NR + q * 512 + ncol],
                        start=(c == 0), stop=(c == HC - 1))
                nc.vector.tensor_copy(sK[0:K, q * 512:q * 512 + ncol],
                                      skp[0:K, 0:ncol])

            # h0 = keys broadcast to (k,b) rows
            h0p = ptile([128, HH], F32)
            nc.tensor.matmul(h0p[0:RP, :], selK16[0:K, :], keys16[0:K, 0:HH],
                             start=True, stop=True, skip_group_check=True)
            nc.tensor.matmul(h0p[64:128, :], selK16[0:K, :], keys16[0:K, HH:H],
                             start=True, stop=True, skip_group_check=True)
            h_cur = p_h.tile([128, HH], F16, tag="h")
            nc.vector.tensor_copy(h_cur[:, :], h0p[:, :])

            tp0 = ptile([128, 3 * 128], F16)
            emit_transposes(h_cur, tp0)
            hT_cur = p_hT.tile([128, 3 * 128], F16, tag="hT")
            nc.vector.tensor_copy(hT_cur[:, :], tp0[:, :])

        # ---- main loop ----
        p_zps = ep(tc.tile_pool(name="zps", bufs=2, space="PSUM"))
        p_gps = ep(tc.tile_pool(name="gps", bufs=2, space="PSUM"))
        p_tps = ep(tc.tile_pool(name="tps", bufs=2, space="PSUM"))
        p_fps = ep(tc.tile_pool(name="fps", bufs=1, space="PSUM"))


        fillP = p_fps.tile([128, HH], F32, tag="fill")

        def inject(zP, t):
            tb, m = (t * BL) // 128, t % 16
            nc.tensor.matmul(zP[0:RP, :], selK16[0:K, :], keyV[0:K, 0:HH],
                             start=True, stop=False, skip_group_check=True)
            nc.tensor.matmul(zP[64:128, :], selK16[0:K, :], keyV[0:K, HH:H],
                             start=True, stop=False, skip_group_check=True)
            lsel = selB32[:, m * RP:(m + 1) * RP]
            nc.tensor.matmul(zP[0:RP, :], lsel,
                             sW[:, tb * H:tb * H + HH],
                             start=False, stop=False, skip_group_check=True)
            nc.tensor.matmul(zP[64:128, :], lsel,
                             sW[:, tb * H + HH:(tb + 1) * H],
                             start=False, stop=False, skip_group_check=True)

        def filler(n):
            for i in range(n):
                nc.tensor.matmul(fillP[0:RP, :], selB32[:, 0:RP], U16[:, 0:HH],
                                 start=True, stop=True, skip_group_check=True)

        zP_next = p_zps.tile([128, HH], F32, tag="z")
        inject(zP_next, 0)

        for t in range(nsteps):
            zP = zP_next
            gqP = p_gps.tile([128, 16], F32, tag="gq")

            # grams: [q_{t-1} | s_t] vs h_{t-1}, split halves over chunks;
            # sK injected into gate cols of the first half mid-group
            for c in range(3):
                mvA = sqT[:, c * BLK + t * 16:c * BLK + t * 16 + 16]
                mvB = sqT[:, (c + 3) * BLK + t * 16:(c + 3) * BLK + t * 16 + 16]
                if c == 2:
                    nc.tensor.matmul(gqP[0:RP, 8:16], selK16[0:K, :],
                                     sK[0:K, t * BL:(t + 1) * BL],
                                     start=False, stop=False,
                                     skip_group_check=True)
                nc.tensor.matmul(gqP[0:RP, 0:16], hts(hT_cur, c), mvA,
                                 start=(c == 0), stop=(c == 2),
                                 skip_group_check=True)
                nc.tensor.matmul(gqP[64:128, 0:16], hts(hT_cur, c + 3), mvB,
                                 start=(c == 0), stop=(c == 2),
                                 skip_group_check=True)

            # z accumulation (injects already done), alternating halves
            for c in range(HC):
                lhs = hts(hT_cur, c)
                nc.tensor.matmul(zP[0:RP, :], lhs, U16[:, c * H:c * H + HH],
                                 start=False, stop=(c == HC - 1),
                                 skip_group_check=True)
                nc.tensor.matmul(zP[64:128, :], lhs,
                                 U16[:, c * H + HH:(c + 1) * H],
                                 start=False, stop=(c == HC - 1),
                                 skip_group_check=True)

            # DVE: masked reduce of [q|g] -> gq2 [128,2]
            gq16 = p_sml.tile([128, 16], F16, tag="gq16")
            gq2 = p_sml.tile([128, 2], F32, tag="gq2")
            nc.vector.tensor_mul(gq16[:, :], gqP[:, :], maskQG[:, :])
            nc.vector.tensor_reduce(gq2[:, :],
                                    gq16[:, :].rearrange("p (a b) -> p a b", a=2),
                                    AXX, ALU.add)
            # PE: cross-half sum [q|g]
            paP = p_gps.tile([128, 2], F32, tag="gq", name="paP")
            nc.tensor.matmul(paP[:, :], P64[:, :], gq2[:, :], start=True, stop=True)

            # next-step injections keep PE busy during the EW chain
            if t + 1 < nsteps:
                zP_next = p_zps.tile([128, HH], F32, tag="z")
                inject(zP_next, t + 1)
            filler(n_fill)

            # ACT: sigmoid, then cand = g*prelu(z)
            gsig = p_sml.tile([128, 1], F32, tag="gsig")
            nc.scalar.activation(gsig[:, :], paP[:, 1:2], ACTF.Sigmoid)
            if t > 0:
                nc.vector.tensor_copy(scores[:, t - 1:t], paP[:, 0:1])
            cand = p_e16.tile([128, HH], F16, tag="cand")
            nc.scalar.activation(cand[:, :], zP[:, :], ACTF.Prelu,
                                 scale=gsig[:, :], alpha=alphav[:, :])
            hu = p_e16.tile([128, HH], F16, tag="hu")
            nc.vector.tensor_add(hu[:, :], h_cur[:, :], cand[:, :])
            # norm^2 (per half), cross-half sum, rsqrt
            squ = p_e16.tile([128, HH], F16, tag="squ")
            ss = p_sml.tile([128, 1], F32, tag="ss")
            nc.vector.tensor_mul(squ[:, :], hu[:, :], hu[:, :])
            nc.vector.tensor_reduce(ss[:, :], squ[:, :], AXX, ALU.add)
            paB = p_gps.tile([128, 1], F32, tag="gq", name="paB")
            nc.tensor.matmul(paB[:, :], P64[:, :], ss[:, :], start=True, stop=True)
            sr = p_sml.tile([128, 1], F32, tag="sr")
            nc.scalar.activation(sr[:, :], paB[:, :], ACTF.Sqrt, bias=epsv[:, :])
            rn = p_sml.tile([128, 1], F32, tag="rn")
            nc.vector.reciprocal_approx_fast(rn[:, :], sr[:, :])
            h_new = p_h.tile([128, HH], F16, tag="h")
            nc.vector.tensor_scalar(h_new[:, :], hu[:, :], rn[:, :], None,
                                    ALU.mult)
            tP = p_tps.tile([128, 3 * 128], F16, tag="t")
            emit_transposes(h_new, tP)
            hT_new = p_hT.tile([128, 3 * 128], F16, tag="hT")
            nc.vector.tensor_copy(hT_new[:, :], tP[:, :])
            h_cur, hT_cur = h_new, hT_new

        # epilogue: last score (q_{T-1} . h_{T-1}) + output head
        gqF = p_gps.tile([128, 16], F32, tag="gq")
        for c in range(3):
            mvA = sqT[:, c * BLK + nsteps * 16:c * BLK + nsteps * 16 + 8]
            mvB = sqT[:, (c + 3) * BLK + nsteps * 16:(c + 3) * BLK + nsteps * 16 + 8]
            nc.tensor.matmul(gqF[0:RP, 0:8], hts(hT_cur, c), mvA,
                             start=(c == 0), stop=(c == 2), skip_group_check=True)
            nc.tensor.matmul(gqF[64:128, 0:8], hts(hT_cur, c + 3), mvB,
                             start=(c == 0), stop=(c == 2), skip_group_check=True)
        gqf16 = p_sml.tile([128, 8], F16, tag="gqf")
        gqf2 = p_sml.tile([128, 1], F32, tag="gqf2")
        nc.vector.tensor_mul(gqf16[:, :], gqF[:, 0:8], maskQG[:, 0:8])
        nc.vector.tensor_reduce(gqf2[:, :], gqf16[:, :], AXX, ALU.add)
        paF = p_gps.tile([128, 2], F32, tag="gq", name="paF")
        nc.tensor.matmul(paF[:, 0:1], P64[:, :], gqf2[:, :], start=True, stop=True)
        nc.vector.tensor_copy(scores[:, nsteps - 1:nsteps], paF[:, 0:1])

        pP = p_fps.tile([128, nsteps], F32, tag="fill", name="head")
        nc.tensor.matmul(pP[0:BL * L, :], Wsel[0:R, 0:BL * L],
                         scores[0:R, 0:nsteps], start=True, stop=True)
        osb = p_prm.tile([128, nsteps], F32, tag="osb")
        nc.vector.tensor_scalar(osb[0:BL * L, :], pP[0:BL * L, :],
                                bvec[0:BL * L, :], None, ALU.add)
        nc.sync.dma_start(bass.AP(d_out, 0, [[1, BL * L], [BL * L, nsteps]]),
                          osb[0:BL * L, :])

    nc.compile()
    return nc


_CACHE = {}


def _get(nsteps):
    if nsteps not in _CACHE:
        _CACHE[nsteps] = _build(nsteps)
    return _CACHE[nsteps]


def run(inputs, **spmd_kwargs):
    nsteps = inputs["features_sentence"].shape[0]
    nc = _get(nsteps)
    consts = _host_consts()
    fs = np.ascontiguousarray(np.asarray(inputs["features_sentence"], dtype=np.float32))
    fe = np.ascontiguousarray(np.asarray(inputs["features_entity"], dtype=np.float32))
    shared = {k: np.ascontiguousarray(np.asarray(inputs[k], dtype=np.float32))
              for k in ("keys", "U", "V", "W", "alpha", "W_out", "b_out")}
    shared.update(consts)
    in_maps = []
    for c in range(NC):
        m = dict(shared)
        m["features_sentence"] = np.ascontiguousarray(fs[:, c * BL:(c + 1) * BL, :])
        m["features_entity"] = np.ascontiguousarray(fe[:, c * BL:(c + 1) * BL, :])
        in_maps.append(m)
    res = run_bass_kernel_spmd(nc, in_maps, core_ids=list(range(NC)), **spmd_kwargs)
    outs = [r["preds"].reshape(nsteps, BL, L) for r in res.results]
    return np.concatenate(outs, axis=1).reshape(nsteps * B, L), res


def kernel(**inputs):
    out, _ = run(inputs)
    return out
